# revision 28
# baseline (speedup 1.0000x reference)
"""Trainium2 Bass kernel for batched scaled-dot-product attention.

Problem (all fp32):
    q = queries @ Wq + bq          [B=4, N=4096, E=64]   (D_MODEL=768)
    k = keys    @ Wk + bk
    v = values  @ Wv + bv
    out = softmax(q k^T / sqrt(E)) @ v                    [B, N, 64]

Sharding: 8 cores, data-parallel over batch x query-half.  Core c handles
batch b=c//2, query rows [h*2048, (h+1)*2048) with h=c%2; it loads the full
keys/values for its batch (softmax needs every key).

Numerics (unchanged from the 129us baseline; rel err ~9e-4):
  * fp16 everywhere on chip; inputs staged feature-major [128, CH, seq];
    q pre-scaled by 1/sqrt(E); v projected x-stationary straight into
    natural layout va [128, 32, 66] with two ones columns so row sums fall
    out of the AV matmul; outputs leave unnormalized, host divides by the
    ones-row sum, transposes and adds bv (exact: softmax weights sum to 1).

Schedule (118.6us vs the 129us baseline, rebuilt from NTFF trace analysis):
  * The old kernel was input-DMA-gated: a 6-buffer staging pool meant the
    last x chunks could not even ISSUE until t=48us, and 2KB descriptors
    made each issue cost ~1-5.6us of serial Sync-engine SWDGE time.  Now
    every chunk is host-packed chunk-contiguous ([128, CH, w] per chunk,
    one ~3-6KB descriptor per partition) and all chunks are SBUF-resident
    (18MB), so all 26 dma_starts (gpsimd SWDGE queue) issue back-to-back
    from t~8us and the 16.6MB stream runs at the full ~360GB/s, done by
    ~50us.  k/v chunks interleave in consumption order (256-col edge
    chunks, 512 in the middle); q blocks 2,3 land mid-pass.
  * 12 full-contraction warmup matmuls draw real power so the HAM clock
    gate lifts at ~12.5us (vs ~21us), putting the whole prologue
    projection chain at 2.4GHz.
  * One fused 64-unit stream (unit u: pass p=u//32, k-tile kt=u%32):
    S^T [128,1024] via 2 matmuls -> one wide exact exp on the scalar
    engine (the structural bottleneck: 64 x ~1.1us) -> 2 AV matmuls
    issued 2 units behind.  The PSUM s-ring (2x2 banks) is shared across
    both passes so the scalar engine never drains at the pass boundary;
    oT pools for pass 1/2 hand off banks mid-stream (2+2, with the 2-bank
    projection pool closed just before).
  * Pass 1 is PE-oversubscribed (all projections live there), pass 2 is
    ACT-bound with ~200ns/unit of PE slack: the last N_DEFER AV pairs of
    pass 1 are deferred (their exp outputs kept alive in SBUF) and
    drained one per even unit during pass 2.  Projection tasks are placed
    at the earliest unit where their chunk has landed, leaving units
    25-31 pure attention so the exp stream runs dense into the boundary.
  * Tail: the last unit's exp is split in halves (block 3 first) and the
    two epilogue copies run on DVE and the then-idle scalar engine in
    parallel, into one [MA, 2, BLK] tile -> single DMA issue per pair.
"""

import numpy as np

B, N, D, E = 4, 4096, 768, 64
NCORES = 8
HALF = N // 2          # query rows per core
CH = D // 128          # 6 feature chunks of the contraction dim
KT = N // 128          # 32 key tiles
BLK = 512              # query block (one PSUM bank of fp32)
SCALE = 1.0 / 8.0      # 1/sqrt(E)
MA = E + 2             # va stationary width (v + two ones columns)
WARMUP_MMS = 8         # K=128 N=512 dummies: real load, ramps the PE clock
                       # to 2.4GHz before the first projections
N_DEFER = 14           # pass-1 AV pairs deferred into pass-2's PE slack

# k/v chunk widths (cols of the 4096 seq): fine at the edges (fast start,
# fine-grained tail deadlines), coarse in the middle.
KV_W = [256, 256, 512, 512, 512, 512, 512, 512, 256, 256]
KV_COL0 = [0]
for _w in KV_W:
    KV_COL0.append(KV_COL0[-1] + _w)
KV_COL0 = KV_COL0[:-1]
NARROW = [i for i, w in enumerate(KV_W) if w == 256]   # -> x_*_s rows
WIDE = [i for i, w in enumerate(KV_W) if w == 512]     # -> x_*_m rows

_CACHE = {}


def _build():
    from contextlib import ExitStack

    import concourse.mybir as mybir
    import concourse.tile as tile
    from concourse import bacc

    f32 = mybir.dt.float32
    f16 = mybir.dt.float16
    EXP = mybir.ActivationFunctionType.Exp

    nc = bacc.Bacc(trn_type="TRN2")
    x_q = nc.dram_tensor("x_q", [4, 128, CH, BLK], f16, kind="ExternalInput")
    x_k_s = nc.dram_tensor("x_k_s", [len(NARROW), 128, CH, 256], f16,
                           kind="ExternalInput")
    x_k_m = nc.dram_tensor("x_k_m", [len(WIDE), 128, CH, 512], f16,
                           kind="ExternalInput")
    x_v_s = nc.dram_tensor("x_v_s", [len(NARROW), 128, CH, 256], f16,
                           kind="ExternalInput")
    x_v_m = nc.dram_tensor("x_v_m", [len(WIDE), 128, CH, 512], f16,
                           kind="ExternalInput")
    w_all = nc.dram_tensor("w_all", [128, 3, CH, E], f16, kind="ExternalInput")
    b_all = nc.dram_tensor("b_all", [E, 3], f32, kind="ExternalInput")

    out = nc.dram_tensor("out", [MA, 4, BLK], f32, kind="ExternalOutput")

    with tile.TileContext(nc) as tc, ExitStack() as ctx:
        singles = ctx.enter_context(tc.tile_pool(name="singles", bufs=1))

        # ---- resident input chunk tiles ----
        xq_t = [singles.tile([128, CH, BLK], f16, name=f"xq{j}")
                for j in range(4)]
        xk_t, xv_t = [], []
        for i, w in enumerate(KV_W):
            xk_t.append(singles.tile([128, CH, w], f16, name=f"xk{i}"))
            xv_t.append(singles.tile([128, CH, w], f16, name=f"xv{i}"))
        w_sb = singles.tile([128, 3, CH, E], f16)
        b_sb = singles.tile([E, 3], f32)

        def kv_dram(which, i):
            s, m = (x_k_s, x_k_m) if which == "k" else (x_v_s, x_v_m)
            if KV_W[i] == 256:
                return s[NARROW.index(i)]
            return m[WIDE.index(i)]

        # ---- DMA issue order == arrival order == consumption order ----
        # Issued from the gpsimd SWDGE queue: its instruction stream loads
        # first (~3us), so bytes start flowing ~4us before the Sync engine
        # could even issue.
        def issue_kv(i):
            nc.gpsimd.dma_start(out=xk_t[i], in_=kv_dram("k", i))
            nc.gpsimd.dma_start(out=xv_t[i], in_=kv_dram("v", i))

        nc.gpsimd.dma_start(out=w_sb, in_=w_all[:, :, :, :])
        nc.gpsimd.dma_start(out=b_sb, in_=b_all[:, :])
        nc.gpsimd.dma_start(out=xq_t[0], in_=x_q[0])
        nc.gpsimd.dma_start(out=xq_t[1], in_=x_q[1])
        for i in range(6):
            issue_kv(i)
        nc.gpsimd.dma_start(out=xq_t[2], in_=x_q[2])
        issue_kv(6)
        nc.gpsimd.dma_start(out=xq_t[3], in_=x_q[3])
        for i in range(7, 10):
            issue_kv(i)

        bqs_sb = singles.tile([E, 1], f32)
        nc.scalar.mul(bqs_sb, b_sb[:, 0:1], SCALE)  # bq / sqrt(E)

        qT = singles.tile([E, HALF], f16)       # q^T / sqrt(E)
        kT = singles.tile([E, N], f16)          # k^T
        va = singles.tile([128, KT, MA], f16)   # v natural + two ones columns
        nc.vector.memset(va[:, :, E:], 1.0)

        # preload the Exp table off the critical path (no DMA dependency)
        warm_col = singles.tile([128, 1], f32)
        nc.vector.memset(warm_col, 0.0)
        dummy = singles.tile([128, 1], f32)
        nc.scalar.activation(dummy, warm_col, EXP)
        warm_big = singles.tile([128, BLK], f16)
        nc.vector.memset(warm_big, 0.01)

        # enough buffers to keep N_DEFER deferred exp outputs alive plus the
        # normal 3-deep pipeline
        pT_pool = ctx.enter_context(tc.tile_pool(name="pT", bufs=N_DEFER + 4))
        ep01 = singles.tile([MA, 2, BLK], f32)
        ep23 = singles.tile([MA, 2, BLK], f32)

        # ---- projections (chunk-granular) ----
        def proj_q(pool, j):
            """q block j (512 cols) -> qT[:, 512j:512j+512], scaled."""
            ps = pool.tile([E, BLK], f32, tag="pj", name="ps")
            for c in range(CH):
                nc.tensor.matmul(ps, lhsT=w_sb[:, 0, c, :],
                                 rhs=xq_t[j][:, c, :],
                                 start=(c == 0), stop=(c == CH - 1))
            nc.vector.tensor_scalar(
                qT[:, j * BLK:(j + 1) * BLK], ps, SCALE, bqs_sb,
                mybir.AluOpType.mult, mybir.AluOpType.add)

        def proj_k(pool, i):
            """k chunk i -> kT[:, c0:c0+w], biased."""
            c0, w = KV_COL0[i], KV_W[i]
            ps = pool.tile([E, BLK], f32, tag="pj", name="ps")
            for c in range(CH):
                nc.tensor.matmul(ps[:, :w], lhsT=w_sb[:, 1, c, :],
                                 rhs=xk_t[i][:, c, :],
                                 start=(c == 0), stop=(c == CH - 1))
            nc.vector.tensor_scalar(
                kT[:, c0:c0 + w], ps[:, :w], b_sb[:, 1:2], None,
                mybir.AluOpType.add)

        def proj_v(pool, kt):
            """x-stationary projection of one 128-row v tile straight into
            va[:, kt] (natural layout, no PE transpose needed)."""
            col = kt * 128
            i = max(j for j in range(len(KV_W)) if KV_COL0[j] <= col)
            sub = (col - KV_COL0[i]) // 128
            ps = pool.tile([128, E], f32, tag="pj", name="psv")
            for c in range(CH):
                nc.tensor.matmul(
                    ps, lhsT=xv_t[i][:, c, sub * 128:(sub + 1) * 128],
                    rhs=w_sb[:, 2, c, :],
                    start=(c == 0), stop=(c == CH - 1))
            nc.vector.tensor_copy(va[:, kt, 0:E], ps)

        # ---- attention stream pieces ----
        def s_exp(s_pool, u, split=False):
            blk_lo = 2 * (u // 32)
            kt = u % 32
            s2 = s_pool.tile([128, 2 * BLK], f32, tag="s", name="s2")
            for i in range(2):
                nc.tensor.matmul(
                    s2[:, i * BLK:(i + 1) * BLK],
                    lhsT=kT[:, kt * 128:(kt + 1) * 128],
                    rhs=qT[:, (blk_lo + i) * BLK:(blk_lo + i + 1) * BLK],
                    start=True, stop=True, skip_group_check=True)
            pT2 = pT_pool.tile([128, 2 * BLK], f16, tag="pT")
            if split:
                # last unit: expose block hi's exp first so the tail AV /
                # copy / DMA chain starts ~0.8us earlier
                nc.scalar.activation(pT2[:, BLK:], s2[:, BLK:], EXP)
                nc.scalar.activation(pT2[:, :BLK], s2[:, :BLK], EXP)
            else:
                nc.scalar.activation(pT2, s2, EXP)
            return pT2

        def av(u, pT2, oT, first, last, rev=False):
            kt = u % 32
            for i in ((1, 0) if rev else (0, 1)):
                nc.tensor.matmul(
                    oT[i],
                    lhsT=va[:, kt, :],
                    rhs=pT2[:, i * BLK:(i + 1) * BLK],
                    start=first, stop=last, skip_group_check=True)

        def epilogue(ep_sb, pair, oT_pair):
            """Copy both oT banks of a block-pair and DMA them out in one
            issue (out[2p:2p+2])."""
            nc.vector.tensor_copy(ep_sb[:, 1, :], oT_pair[1])
            nc.vector.tensor_copy(ep_sb[:, 0, :], oT_pair[0])
            nc.sync.dma_start(out=out[:, 2 * pair:2 * pair + 2, :], in_=ep_sb)

        # ---- PSUM layout: s-ring first (banks 0-3), rest hands off ----
        s_pool = ctx.enter_context(tc.tile_pool(name="s", bufs=2,
                                                space="PSUM"))

        # ================= prologue =================
        from contextlib import ExitStack as _ES

        with _ES() as pro:
            warm_ps = pro.enter_context(
                tc.tile_pool(name="warm", bufs=1, space="PSUM"))
            pjq = pro.enter_context(
                tc.tile_pool(name="pjq", bufs=2, space="PSUM"))
            wp = warm_ps.tile([128, BLK], f32, tag="w", name="wp")
            for _ in range(WARMUP_MMS):
                nc.tensor.matmul(wp, lhsT=warm_big[:, 0:128], rhs=warm_big,
                                 start=True, stop=True, skip_group_check=True)
            proj_q(pjq, 0)
            proj_q(pjq, 1)
            proj_k(pjq, 0)

        # ======== main stream: 64 units, deferred-AV rebalance ========
        # Units 0-31: q-blocks 0,1 + all streaming projections (pass 1 is
        # PE-oversubscribed).  The AV pairs of units 22-31 are deferred and
        # drained one per even unit during 32-50, where pass 2 is ACT-bound
        # and the PE has slack.  v-projections run 2 tiles per visit so the
        # chain-entry latency is amortized.
        # Task units chosen so each projection runs at the earliest unit
        # where its chunk has landed (DMA arrival-driven), leaving units
        # 25-31 as pure attention so the exp stream runs dense there.
        k_sched = {1: 0, 2: 1, 3: 4, 4: 8, 5: 11, 6: 14, 7: 18, 8: 20, 9: 21}
        # v-projection visits: (unit, first tile, n tiles).  Tiles >= 18
        # have deferred AVs, so their projections move past the k tasks
        # into the quiet units 25-30.
        v_sched = [(0, 0, 2), (2, 2, 2), (4, 4, 2), (5, 6, 2), (7, 8, 2),
                   (9, 10, 4), (12, 14, 4), (25, 18, 4), (27, 22, 4),
                   (28, 26, 4), (30, 30, 2)]
        u_defer0 = 32 - N_DEFER          # first deferred unit (22)
        with _ES() as main_sc:
            o1 = main_sc.enter_context(tc.tile_pool(name="o1", bufs=1,
                                                    space="PSUM"))
            oT01 = [o1.tile([MA, BLK], f32, tag=f"oT{i}", name=f"oT{i}")
                    for i in range(2)]
            pT_hist = {}

            with _ES() as p1:
                pj1 = p1.enter_context(tc.tile_pool(name="pj1", bufs=2,
                                                    space="PSUM"))
                pend = {}
                for i, u in k_sched.items():
                    pend.setdefault(u, []).append(lambda i=i: proj_k(pj1, i))
                for u, kt0, n in v_sched:
                    pend.setdefault(u, []).append(
                        lambda kt0=kt0, n=n: [proj_v(pj1, kt)
                                              for kt in range(kt0, kt0 + n)])
                pend.setdefault(12, []).append(lambda: proj_q(pj1, 2))
                pend.setdefault(16, []).append(lambda: proj_q(pj1, 3))

                for u in range(32):
                    pT_hist[u] = s_exp(s_pool, u)
                    for fn in pend.pop(u, ()):
                        fn()
                    if 2 <= u and u - 2 < u_defer0:
                        av(u - 2, pT_hist.pop(u - 2), oT01,
                           first=(u - 2 == 0), last=False)

            # pj1 closed -> banks 6,7 free for o2
            o2 = main_sc.enter_context(tc.tile_pool(name="o2", bufs=1,
                                                    space="PSUM"))
            oT23 = [o2.tile([MA, BLK], f32, tag=f"oT{i + 2}",
                            name=f"oT{i + 2}") for i in range(2)]

            drain_at = {du: u_defer0 + j for j, du in enumerate(
                [32, 33] + list(range(34, 34 + 2 * (N_DEFER - 2), 2)))}
            for u in range(32, 64):
                # drain deferred pass-1 AV pairs BEFORE this unit's S pair:
                # their inputs are long ready, so the PE chews them while
                # the s-ring semaphore from the exp stream resolves
                if u in drain_at:
                    du = drain_at[u]
                    av(du, pT_hist.pop(du), oT01, first=False,
                       last=(du == 31), rev=(du == 31))
                    if du == 31:
                        epilogue(ep01, 0, oT01)
                pT_hist[u] = s_exp(s_pool, u, split=(u == 63))
                if 34 <= u:
                    av(u - 2, pT_hist.pop(u - 2), oT23,
                       first=(u - 2 == 32), last=False)
            av(62, pT_hist.pop(62), oT23, first=False, last=False)
            av(63, pT_hist.pop(63), oT23, first=False, last=True, rev=True)
            # tail: block-3 copy on DVE, block-2 copy on the now-idle scalar
            # engine, one DMA issue for both
            nc.vector.tensor_copy(ep23[:, 1, :], oT23[1])
            nc.sync.dma_start(out=out[:, 3:4, :], in_=ep23[:, 1:2, :])
            nc.scalar.copy(ep23[:, 0, :], oT23[0])
            nc.gpsimd.dma_start(out=out[:, 2:3, :], in_=ep23[:, 0:1, :])

    nc.finalize()
    return nc


def get_nc():
    if "nc" not in _CACHE:
        _CACHE["nc"] = _build()
    return _CACHE["nc"]


def _feat_major(x2d):
    """[seq, D] fp32 -> [128, CH, seq] fp16 (feature-major, chunked)."""
    xT = np.ascontiguousarray(x2d.T)                 # [D, seq]
    xT = xT.reshape(CH, 128, -1).transpose(1, 0, 2)  # [128, CH, seq]
    return np.ascontiguousarray(xT).astype(np.float16)


def _kv_pack(fm):
    """[128, CH, 4096] -> (narrow [n,128,CH,256], wide [m,128,CH,512])."""
    nar = np.stack([fm[:, :, KV_COL0[i]:KV_COL0[i] + 256] for i in NARROW])
    wid = np.stack([fm[:, :, KV_COL0[i]:KV_COL0[i] + 512] for i in WIDE])
    return np.ascontiguousarray(nar), np.ascontiguousarray(wid)


def make_in_maps(queries, keys, values, Wq, bq, Wk, bk, Wv, bv):
    def w_prep(w):
        w = np.asarray(w, np.float32).reshape(CH, 128, E)
        return w.transpose(1, 0, 2).astype(np.float16)  # [128, CH, E]

    w_all = np.ascontiguousarray(
        np.stack([w_prep(Wq), w_prep(Wk), w_prep(Wv)], axis=1))
    b_all = np.ascontiguousarray(
        np.stack([bq, bk, bv], axis=1).astype(np.float32))
    shared = {"w_all": w_all, "b_all": b_all}

    queries = np.asarray(queries, np.float32)
    keys = np.asarray(keys, np.float32)
    values = np.asarray(values, np.float32)
    kv_cache = {}
    in_maps = []
    for c in range(NCORES):
        b, h = divmod(c, 2)
        if b not in kv_cache:
            ks, km = _kv_pack(_feat_major(keys[b]))
            vs, vm = _kv_pack(_feat_major(values[b]))
            kv_cache[b] = (ks, km, vs, vm)
        ks, km, vs, vm = kv_cache[b]
        fq = _feat_major(queries[b, h * HALF:(h + 1) * HALF, :])
        xq = np.ascontiguousarray(
            np.stack([fq[:, :, j * BLK:(j + 1) * BLK] for j in range(4)]))
        in_maps.append({
            "x_q": xq,
            "x_k_s": ks, "x_k_m": km,
            "x_v_s": vs, "x_v_m": vm,
            **shared,
        })
    return in_maps


def run(trace=False, **inputs):
    from concourse.bass_utils import run_bass_kernel_spmd

    nc = get_nc()
    in_maps = make_in_maps(**inputs)
    res = run_bass_kernel_spmd(
        nc, in_maps, core_ids=list(range(NCORES)), trace=trace)
    bv = np.asarray(inputs["bv"], np.float32)
    full = np.empty((B, N, E), dtype=np.float32)
    for c in range(NCORES):
        b, h = divmod(c, 2)
        oT = res.results[c]["out"].transpose(1, 0, 2)   # [4, MA, BLK]
        o = oT[:, :E, :] / oT[:, E:E + 1, :]            # normalize
        o = o.transpose(0, 2, 1).reshape(HALF, E) + bv  # [2048, 64]
        full[b, h * HALF:(h + 1) * HALF, :] = o
    return full, res


def kernel(**inputs):
    full, _ = run(trace=False, **inputs)
    return full


# revision 29
# speedup vs baseline: 1.0160x; 1.0160x over previous
"""Trainium2 Bass kernel for batched scaled-dot-product attention.

Problem (all fp32):
    q = queries @ Wq + bq          [B=4, N=4096, E=64]   (D_MODEL=768)
    k = keys    @ Wk + bk
    v = values  @ Wv + bv
    out = softmax(q k^T / sqrt(E)) @ v                    [B, N, 64]

Sharding: 8 cores, data-parallel over batch x query-half.  Core c handles
batch b=c//2, query rows [h*2048, (h+1)*2048) with h=c%2; it loads the full
keys/values for its batch (softmax needs every key).

Numerics (unchanged from the 129us baseline; rel err ~9e-4):
  * fp16 everywhere on chip; inputs staged feature-major [128, CH, seq];
    q pre-scaled by 1/sqrt(E); v projected x-stationary straight into
    natural layout va [128, 32, 66] with two ones columns so row sums fall
    out of the AV matmul; outputs leave unnormalized, host divides by the
    ones-row sum, transposes and adds bv (exact: softmax weights sum to 1).

Schedule (118.6us vs the 129us baseline, rebuilt from NTFF trace analysis):
  * The old kernel was input-DMA-gated: a 6-buffer staging pool meant the
    last x chunks could not even ISSUE until t=48us, and 2KB descriptors
    made each issue cost ~1-5.6us of serial Sync-engine SWDGE time.  Now
    every chunk is host-packed chunk-contiguous ([128, CH, w] per chunk,
    one ~3-6KB descriptor per partition) and all chunks are SBUF-resident
    (18MB), so all 26 dma_starts (gpsimd SWDGE queue) issue back-to-back
    from t~8us and the 16.6MB stream runs at the full ~360GB/s, done by
    ~50us.  k/v chunks interleave in consumption order (256-col edge
    chunks, 512 in the middle); q blocks 2,3 land mid-pass.
  * 12 full-contraction warmup matmuls draw real power so the HAM clock
    gate lifts at ~12.5us (vs ~21us), putting the whole prologue
    projection chain at 2.4GHz.
  * One fused 64-unit stream (unit u: pass p=u//32, k-tile kt=u%32):
    S^T [128,1024] via 2 matmuls -> one wide exact exp on the scalar
    engine (the structural bottleneck: 64 x ~1.1us) -> 2 AV matmuls
    issued 2 units behind.  The PSUM s-ring (2x2 banks) is shared across
    both passes so the scalar engine never drains at the pass boundary;
    oT pools for pass 1/2 hand off banks mid-stream (2+2, with the 2-bank
    projection pool closed just before).
  * Pass 1 is PE-oversubscribed (all projections live there), pass 2 is
    ACT-bound with ~200ns/unit of PE slack: the last N_DEFER AV pairs of
    pass 1 are deferred (their exp outputs kept alive in SBUF) and
    drained one per even unit during pass 2.  Projection tasks are placed
    at the earliest unit where their chunk has landed, leaving units
    25-31 pure attention so the exp stream runs dense into the boundary.
  * Tail: the last unit's exp is split in halves (block 3 first) and the
    two epilogue copies run on DVE and the then-idle scalar engine in
    parallel, into one [MA, 2, BLK] tile -> single DMA issue per pair.
"""

import numpy as np

B, N, D, E = 4, 4096, 768, 64
NCORES = 8
HALF = N // 2          # query rows per core
CH = D // 128          # 6 feature chunks of the contraction dim
KT = N // 128          # 32 key tiles
BLK = 512              # query block (one PSUM bank of fp32)
SCALE = 1.0 / 8.0      # 1/sqrt(E)
MA = E + 2             # va stationary width (v + two ones columns)
WARMUP_MMS = 12        # K=128 N=512 dummies: real load, ramps the PE clock
                       # to 2.4GHz before the first projections
N_DEFER = 14           # pass-1 AV pairs deferred into pass-2's PE slack

# k/v chunk widths (cols of the 4096 seq): fine at the edges (fast start,
# fine-grained tail deadlines), coarse in the middle.
KV_W = [256, 256, 512, 512, 512, 512, 512, 512, 256, 256]
KV_COL0 = [0]
for _w in KV_W:
    KV_COL0.append(KV_COL0[-1] + _w)
KV_COL0 = KV_COL0[:-1]
NARROW = [i for i, w in enumerate(KV_W) if w == 256]   # -> x_*_s rows
WIDE = [i for i, w in enumerate(KV_W) if w == 512]     # -> x_*_m rows

_CACHE = {}


def _build():
    from contextlib import ExitStack

    import concourse.mybir as mybir
    import concourse.tile as tile
    from concourse import bacc

    f32 = mybir.dt.float32
    f16 = mybir.dt.float16
    EXP = mybir.ActivationFunctionType.Exp

    nc = bacc.Bacc(trn_type="TRN2")
    x_q = nc.dram_tensor("x_q", [4, 128, CH, BLK], f16, kind="ExternalInput")
    x_k_s = nc.dram_tensor("x_k_s", [len(NARROW), 128, CH, 256], f16,
                           kind="ExternalInput")
    x_k_m = nc.dram_tensor("x_k_m", [len(WIDE), 128, CH, 512], f16,
                           kind="ExternalInput")
    x_v_s = nc.dram_tensor("x_v_s", [len(NARROW), 128, CH, 256], f16,
                           kind="ExternalInput")
    x_v_m = nc.dram_tensor("x_v_m", [len(WIDE), 128, CH, 512], f16,
                           kind="ExternalInput")
    w_all = nc.dram_tensor("w_all", [128, 3, CH, E], f16, kind="ExternalInput")
    b_all = nc.dram_tensor("b_all", [E, 3], f32, kind="ExternalInput")

    out = nc.dram_tensor("out", [MA, 4, BLK], f32, kind="ExternalOutput")

    with tile.TileContext(nc) as tc, ExitStack() as ctx:
        singles = ctx.enter_context(tc.tile_pool(name="singles", bufs=1))

        # ---- resident input chunk tiles ----
        xq_t = [singles.tile([128, CH, BLK], f16, name=f"xq{j}")
                for j in range(4)]
        xk_t, xv_t = [], []
        for i, w in enumerate(KV_W):
            xk_t.append(singles.tile([128, CH, w], f16, name=f"xk{i}"))
            xv_t.append(singles.tile([128, CH, w], f16, name=f"xv{i}"))
        w_sb = singles.tile([128, 3, CH, E], f16)
        b_sb = singles.tile([E, 3], f32)

        def kv_dram(which, i):
            s, m = (x_k_s, x_k_m) if which == "k" else (x_v_s, x_v_m)
            if KV_W[i] == 256:
                return s[NARROW.index(i)]
            return m[WIDE.index(i)]

        # ---- DMA issue order == arrival order == consumption order ----
        # Issued from the gpsimd SWDGE queue: its instruction stream loads
        # first (~3us), so bytes start flowing ~4us before the Sync engine
        # could even issue.
        def issue_kv(i):
            nc.gpsimd.dma_start(out=xk_t[i], in_=kv_dram("k", i))
            nc.gpsimd.dma_start(out=xv_t[i], in_=kv_dram("v", i))

        nc.gpsimd.dma_start(out=w_sb, in_=w_all[:, :, :, :])
        nc.gpsimd.dma_start(out=b_sb, in_=b_all[:, :])
        nc.gpsimd.dma_start(out=xq_t[0], in_=x_q[0])
        nc.gpsimd.dma_start(out=xq_t[1], in_=x_q[1])
        for i in range(6):
            issue_kv(i)
        nc.gpsimd.dma_start(out=xq_t[2], in_=x_q[2])
        issue_kv(6)
        nc.gpsimd.dma_start(out=xq_t[3], in_=x_q[3])
        for i in range(7, 10):
            issue_kv(i)

        bqs_sb = singles.tile([E, 1], f32)
        nc.scalar.mul(bqs_sb, b_sb[:, 0:1], SCALE)  # bq / sqrt(E)

        qT = singles.tile([E, HALF], f16)       # q^T / sqrt(E)
        kT = singles.tile([E, N], f16)          # k^T
        va = singles.tile([128, KT, MA], f16)   # v natural + two ones columns
        nc.vector.memset(va[:, :, E:], 1.0)

        # preload the Exp table off the critical path (no DMA dependency)
        warm_col = singles.tile([128, 1], f32)
        nc.vector.memset(warm_col, 0.0)
        dummy = singles.tile([128, 1], f32)
        nc.scalar.activation(dummy, warm_col, EXP)
        warm_big = singles.tile([128, BLK], f16)
        nc.vector.memset(warm_big, 0.01)

        # enough buffers to keep N_DEFER deferred exp outputs alive plus the
        # normal pipeline AND slack, so the exp stream is never in lockstep
        # with the pass-2 drain schedule through the pool ring
        pT_pool = ctx.enter_context(tc.tile_pool(name="pT", bufs=N_DEFER + 8))
        ep01 = singles.tile([MA, 2, BLK], f32)
        ep23 = singles.tile([MA, 2, BLK], f32)

        # ---- projections (chunk-granular) ----
        def proj_q(pool, j):
            """q block j (512 cols) -> qT[:, 512j:512j+512], scaled."""
            ps = pool.tile([E, BLK], f32, tag="pj", name="ps")
            for c in range(CH):
                nc.tensor.matmul(ps, lhsT=w_sb[:, 0, c, :],
                                 rhs=xq_t[j][:, c, :],
                                 start=(c == 0), stop=(c == CH - 1))
            nc.vector.tensor_scalar(
                qT[:, j * BLK:(j + 1) * BLK], ps, SCALE, bqs_sb,
                mybir.AluOpType.mult, mybir.AluOpType.add)

        def proj_k(pool, i):
            """k chunk i -> kT[:, c0:c0+w], biased."""
            c0, w = KV_COL0[i], KV_W[i]
            ps = pool.tile([E, BLK], f32, tag="pj", name="ps")
            for c in range(CH):
                nc.tensor.matmul(ps[:, :w], lhsT=w_sb[:, 1, c, :],
                                 rhs=xk_t[i][:, c, :],
                                 start=(c == 0), stop=(c == CH - 1))
            nc.vector.tensor_scalar(
                kT[:, c0:c0 + w], ps[:, :w], b_sb[:, 1:2], None,
                mybir.AluOpType.add)

        def proj_v(pool, kt):
            """x-stationary projection of one 128-row v tile straight into
            va[:, kt] (natural layout, no PE transpose needed)."""
            col = kt * 128
            i = max(j for j in range(len(KV_W)) if KV_COL0[j] <= col)
            sub = (col - KV_COL0[i]) // 128
            ps = pool.tile([128, E], f32, tag="pj", name="psv")
            for c in range(CH):
                nc.tensor.matmul(
                    ps, lhsT=xv_t[i][:, c, sub * 128:(sub + 1) * 128],
                    rhs=w_sb[:, 2, c, :],
                    start=(c == 0), stop=(c == CH - 1))
            nc.vector.tensor_copy(va[:, kt, 0:E], ps)

        # ---- attention stream pieces ----
        def s_exp(s_pool, u, split=False):
            blk_lo = 2 * (u // 32)
            kt = u % 32
            s2 = s_pool.tile([128, 2 * BLK], f32, tag="s", name="s2")
            for i in range(2):
                nc.tensor.matmul(
                    s2[:, i * BLK:(i + 1) * BLK],
                    lhsT=kT[:, kt * 128:(kt + 1) * 128],
                    rhs=qT[:, (blk_lo + i) * BLK:(blk_lo + i + 1) * BLK],
                    start=True, stop=True, skip_group_check=True)
            pT2 = pT_pool.tile([128, 2 * BLK], f16, tag="pT")
            if split:
                # last unit: expose block hi's exp first so the tail AV /
                # copy / DMA chain starts ~0.8us earlier
                nc.scalar.activation(pT2[:, BLK:], s2[:, BLK:], EXP)
                nc.scalar.activation(pT2[:, :BLK], s2[:, :BLK], EXP)
            else:
                nc.scalar.activation(pT2, s2, EXP)
            return pT2

        def av(u, pT2, oT, first, last, rev=False):
            kt = u % 32
            for i in ((1, 0) if rev else (0, 1)):
                nc.tensor.matmul(
                    oT[i],
                    lhsT=va[:, kt, :],
                    rhs=pT2[:, i * BLK:(i + 1) * BLK],
                    start=first, stop=last, skip_group_check=True)

        def epilogue(ep_sb, pair, oT_pair):
            """Copy both oT banks of a block-pair and DMA them out in one
            issue (out[2p:2p+2])."""
            nc.vector.tensor_copy(ep_sb[:, 1, :], oT_pair[1])
            nc.vector.tensor_copy(ep_sb[:, 0, :], oT_pair[0])
            nc.sync.dma_start(out=out[:, 2 * pair:2 * pair + 2, :], in_=ep_sb)

        # ---- PSUM layout: s-ring first (banks 0-3), rest hands off ----
        s_pool = ctx.enter_context(tc.tile_pool(name="s", bufs=2,
                                                space="PSUM"))

        # ================= prologue =================
        from contextlib import ExitStack as _ES

        with _ES() as pro:
            warm_ps = pro.enter_context(
                tc.tile_pool(name="warm", bufs=1, space="PSUM"))
            pjq = pro.enter_context(
                tc.tile_pool(name="pjq", bufs=2, space="PSUM"))
            wp = warm_ps.tile([128, BLK], f32, tag="w", name="wp")
            for _ in range(WARMUP_MMS):
                nc.tensor.matmul(wp, lhsT=warm_big[:, 0:128], rhs=warm_big,
                                 start=True, stop=True, skip_group_check=True)
            proj_q(pjq, 0)
            proj_q(pjq, 1)
            proj_k(pjq, 0)

        # ======== main stream: 64 units, deferred-AV rebalance ========
        # Units 0-31: q-blocks 0,1 + all streaming projections (pass 1 is
        # PE-oversubscribed).  The AV pairs of units 22-31 are deferred and
        # drained one per even unit during 32-50, where pass 2 is ACT-bound
        # and the PE has slack.  v-projections run 2 tiles per visit so the
        # chain-entry latency is amortized.
        # Task units chosen so each projection runs at the earliest unit
        # where its chunk has landed (DMA arrival-driven), leaving units
        # 25-31 as pure attention so the exp stream runs dense there.
        k_sched = {1: 0, 2: 1, 3: 4, 4: 8, 5: 11, 6: 14, 7: 18, 8: 20, 9: 21}
        # v-projection visits: (unit, first tile, n tiles).  Tiles >= 18
        # have deferred AVs, so their projections move past the k tasks
        # into the quiet units 25-30.
        v_sched = [(0, 0, 2), (2, 2, 2), (4, 4, 2), (5, 6, 2), (7, 8, 2),
                   (9, 10, 4), (12, 14, 4), (25, 18, 4), (27, 22, 4),
                   (28, 26, 4), (30, 30, 2)]
        u_defer0 = 32 - N_DEFER          # first deferred unit (22)
        with _ES() as main_sc:
            o1 = main_sc.enter_context(tc.tile_pool(name="o1", bufs=1,
                                                    space="PSUM"))
            oT01 = [o1.tile([MA, BLK], f32, tag=f"oT{i}", name=f"oT{i}")
                    for i in range(2)]
            pT_hist = {}

            with _ES() as p1:
                pj1 = p1.enter_context(tc.tile_pool(name="pj1", bufs=2,
                                                    space="PSUM"))
                pend = {}
                for i, u in k_sched.items():
                    pend.setdefault(u, []).append(lambda i=i: proj_k(pj1, i))
                for u, kt0, n in v_sched:
                    pend.setdefault(u, []).append(
                        lambda kt0=kt0, n=n: [proj_v(pj1, kt)
                                              for kt in range(kt0, kt0 + n)])
                pend.setdefault(12, []).append(lambda: proj_q(pj1, 2))
                pend.setdefault(16, []).append(lambda: proj_q(pj1, 3))

                for u in range(32):
                    pT_hist[u] = s_exp(s_pool, u)
                    for fn in pend.pop(u, ()):
                        fn()
                    if 2 <= u and u - 2 < u_defer0:
                        av(u - 2, pT_hist.pop(u - 2), oT01,
                           first=(u - 2 == 0), last=False)

            # pj1 closed -> banks 6,7 free for o2
            o2 = main_sc.enter_context(tc.tile_pool(name="o2", bufs=1,
                                                    space="PSUM"))
            oT23 = [o2.tile([MA, BLK], f32, tag=f"oT{i + 2}",
                            name=f"oT{i + 2}") for i in range(2)]

            drain_at = {du: u_defer0 + j for j, du in enumerate(
                [32, 33] + list(range(34, 34 + 2 * (N_DEFER - 2), 2)))}
            for u in range(32, 64):
                # drain deferred pass-1 AV pairs BEFORE this unit's S pair:
                # their inputs are long ready, so the PE chews them while
                # the s-ring semaphore from the exp stream resolves
                if u in drain_at:
                    du = drain_at[u]
                    av(du, pT_hist.pop(du), oT01, first=False,
                       last=(du == 31), rev=(du == 31))
                    if du == 31:
                        epilogue(ep01, 0, oT01)
                pT_hist[u] = s_exp(s_pool, u, split=(u == 63))
                if 34 <= u:
                    av(u - 2, pT_hist.pop(u - 2), oT23,
                       first=(u - 2 == 32), last=False)
            av(62, pT_hist.pop(62), oT23, first=False, last=False)
            av(63, pT_hist.pop(63), oT23, first=False, last=True, rev=True)
            # tail: block-3 copy on DVE, block-2 copy on the now-idle scalar
            # engine, one DMA issue for both
            nc.vector.tensor_copy(ep23[:, 1, :], oT23[1])
            nc.scalar.copy(ep23[:, 0, :], oT23[0])
            nc.sync.dma_start(out=out[:, 2:4, :], in_=ep23)

    nc.finalize()
    return nc


def get_nc():
    if "nc" not in _CACHE:
        _CACHE["nc"] = _build()
    return _CACHE["nc"]


def _feat_major(x2d):
    """[seq, D] fp32 -> [128, CH, seq] fp16 (feature-major, chunked)."""
    xT = np.ascontiguousarray(x2d.T)                 # [D, seq]
    xT = xT.reshape(CH, 128, -1).transpose(1, 0, 2)  # [128, CH, seq]
    return np.ascontiguousarray(xT).astype(np.float16)


def _kv_pack(fm):
    """[128, CH, 4096] -> (narrow [n,128,CH,256], wide [m,128,CH,512])."""
    nar = np.stack([fm[:, :, KV_COL0[i]:KV_COL0[i] + 256] for i in NARROW])
    wid = np.stack([fm[:, :, KV_COL0[i]:KV_COL0[i] + 512] for i in WIDE])
    return np.ascontiguousarray(nar), np.ascontiguousarray(wid)


def make_in_maps(queries, keys, values, Wq, bq, Wk, bk, Wv, bv):
    def w_prep(w):
        w = np.asarray(w, np.float32).reshape(CH, 128, E)
        return w.transpose(1, 0, 2).astype(np.float16)  # [128, CH, E]

    w_all = np.ascontiguousarray(
        np.stack([w_prep(Wq), w_prep(Wk), w_prep(Wv)], axis=1))
    b_all = np.ascontiguousarray(
        np.stack([bq, bk, bv], axis=1).astype(np.float32))
    shared = {"w_all": w_all, "b_all": b_all}

    queries = np.asarray(queries, np.float32)
    keys = np.asarray(keys, np.float32)
    values = np.asarray(values, np.float32)
    kv_cache = {}
    in_maps = []
    for c in range(NCORES):
        b, h = divmod(c, 2)
        if b not in kv_cache:
            ks, km = _kv_pack(_feat_major(keys[b]))
            vs, vm = _kv_pack(_feat_major(values[b]))
            kv_cache[b] = (ks, km, vs, vm)
        ks, km, vs, vm = kv_cache[b]
        fq = _feat_major(queries[b, h * HALF:(h + 1) * HALF, :])
        xq = np.ascontiguousarray(
            np.stack([fq[:, :, j * BLK:(j + 1) * BLK] for j in range(4)]))
        in_maps.append({
            "x_q": xq,
            "x_k_s": ks, "x_k_m": km,
            "x_v_s": vs, "x_v_m": vm,
            **shared,
        })
    return in_maps


def run(trace=False, **inputs):
    from concourse.bass_utils import run_bass_kernel_spmd

    nc = get_nc()
    in_maps = make_in_maps(**inputs)
    res = run_bass_kernel_spmd(
        nc, in_maps, core_ids=list(range(NCORES)), trace=trace)
    bv = np.asarray(inputs["bv"], np.float32)
    full = np.empty((B, N, E), dtype=np.float32)
    for c in range(NCORES):
        b, h = divmod(c, 2)
        oT = res.results[c]["out"].transpose(1, 0, 2)   # [4, MA, BLK]
        o = oT[:, :E, :] / oT[:, E:E + 1, :]            # normalize
        o = o.transpose(0, 2, 1).reshape(HALF, E) + bv  # [2048, 64]
        full[b, h * HALF:(h + 1) * HALF, :] = o
    return full, res


def kernel(**inputs):
    full, _ = run(trace=False, **inputs)
    return full


# revision 30
# speedup vs baseline: 1.0265x; 1.0104x over previous
"""Trainium2 Bass kernel for batched scaled-dot-product attention.

Problem (all fp32):
    q = queries @ Wq + bq          [B=4, N=4096, E=64]   (D_MODEL=768)
    k = keys    @ Wk + bk
    v = values  @ Wv + bv
    out = softmax(q k^T / sqrt(E)) @ v                    [B, N, 64]

Sharding: 8 cores, data-parallel over batch x query-half.  Core c handles
batch b=c//2, query rows [h*2048, (h+1)*2048) with h=c%2; it loads the full
keys/values for its batch (softmax needs every key).

Numerics (unchanged from the 129us baseline; rel err ~9e-4):
  * fp16 everywhere on chip; inputs staged feature-major [128, CH, seq];
    q pre-scaled by 1/sqrt(E); v projected x-stationary straight into
    natural layout va [128, 32, 66] with two ones columns so row sums fall
    out of the AV matmul; outputs leave unnormalized, host divides by the
    ones-row sum, transposes and adds bv (exact: softmax weights sum to 1).

Schedule (118.6us vs the 129us baseline, rebuilt from NTFF trace analysis):
  * The old kernel was input-DMA-gated: a 6-buffer staging pool meant the
    last x chunks could not even ISSUE until t=48us, and 2KB descriptors
    made each issue cost ~1-5.6us of serial Sync-engine SWDGE time.  Now
    every chunk is host-packed chunk-contiguous ([128, CH, w] per chunk,
    one ~3-6KB descriptor per partition) and all chunks are SBUF-resident
    (18MB), so all 26 dma_starts (gpsimd SWDGE queue) issue back-to-back
    from t~8us and the 16.6MB stream runs at the full ~360GB/s, done by
    ~50us.  k/v chunks interleave in consumption order (256-col edge
    chunks, 512 in the middle); q blocks 2,3 land mid-pass.
  * 12 full-contraction warmup matmuls draw real power so the HAM clock
    gate lifts at ~12.5us (vs ~21us), putting the whole prologue
    projection chain at 2.4GHz.
  * One fused 64-unit stream (unit u: pass p=u//32, k-tile kt=u%32):
    S^T [128,1024] via 2 matmuls -> one wide exact exp on the scalar
    engine (the structural bottleneck: 64 x ~1.1us) -> 2 AV matmuls
    issued 2 units behind.  The PSUM s-ring (2x2 banks) is shared across
    both passes so the scalar engine never drains at the pass boundary;
    oT pools for pass 1/2 hand off banks mid-stream (2+2, with the 2-bank
    projection pool closed just before).
  * Pass 1 is PE-oversubscribed (all projections live there), pass 2 is
    ACT-bound with ~200ns/unit of PE slack: the last N_DEFER AV pairs of
    pass 1 are deferred (their exp outputs kept alive in SBUF) and
    drained one per even unit during pass 2.  Projection tasks are placed
    at the earliest unit where their chunk has landed, leaving units
    25-31 pure attention so the exp stream runs dense into the boundary.
  * Tail: the last unit's exp is split in halves (block 3 first) and the
    two epilogue copies run on DVE and the then-idle scalar engine in
    parallel, into one [MA, 2, BLK] tile -> single DMA issue per pair.
"""

import numpy as np

B, N, D, E = 4, 4096, 768, 64
NCORES = 8
HALF = N // 2          # query rows per core
CH = D // 128          # 6 feature chunks of the contraction dim
KT = N // 128          # 32 key tiles
BLK = 512              # query block (one PSUM bank of fp32)
SCALE = 1.0 / 8.0      # 1/sqrt(E)
MA = E + 2             # va stationary width (v + two ones columns)
WARMUP_MMS = 12        # K=128 N=512 dummies: real load, ramps the PE clock
                       # to 2.4GHz before the first projections
N_DEFER = 14           # pass-1 AV pairs deferred into pass-2's PE slack

# k/v chunk widths (cols of the 4096 seq): fine at the edges (fast start,
# fine-grained tail deadlines), coarse in the middle.
KV_W = [256, 256, 512, 512, 512, 512, 512, 512, 256, 256]
KV_COL0 = [0]
for _w in KV_W:
    KV_COL0.append(KV_COL0[-1] + _w)
KV_COL0 = KV_COL0[:-1]
NARROW = [i for i, w in enumerate(KV_W) if w == 256]   # -> x_*_s rows
WIDE = [i for i, w in enumerate(KV_W) if w == 512]     # -> x_*_m rows

_CACHE = {}


def _build():
    from contextlib import ExitStack

    import concourse.mybir as mybir
    import concourse.tile as tile
    from concourse import bacc

    f32 = mybir.dt.float32
    f16 = mybir.dt.float16
    EXP = mybir.ActivationFunctionType.Exp

    nc = bacc.Bacc(trn_type="TRN2")
    x_q = nc.dram_tensor("x_q", [4, 128, CH, BLK], f16, kind="ExternalInput")
    x_k_s = nc.dram_tensor("x_k_s", [len(NARROW), 128, CH, 256], f16,
                           kind="ExternalInput")
    x_k_m = nc.dram_tensor("x_k_m", [len(WIDE), 128, CH, 512], f16,
                           kind="ExternalInput")
    x_v_s = nc.dram_tensor("x_v_s", [len(NARROW), 128, CH, 256], f16,
                           kind="ExternalInput")
    x_v_m = nc.dram_tensor("x_v_m", [len(WIDE), 128, CH, 512], f16,
                           kind="ExternalInput")
    w_all = nc.dram_tensor("w_all", [128, 3, CH, E], f16, kind="ExternalInput")
    b_all = nc.dram_tensor("b_all", [E, 3], f32, kind="ExternalInput")

    out = nc.dram_tensor("out", [MA, 4, BLK], f32, kind="ExternalOutput")

    with tile.TileContext(nc) as tc, ExitStack() as ctx:
        singles = ctx.enter_context(tc.tile_pool(name="singles", bufs=1))

        # ---- resident input chunk tiles ----
        xq_t = [singles.tile([128, CH, BLK], f16, name=f"xq{j}")
                for j in range(4)]
        xk_t, xv_t = [], []
        for i, w in enumerate(KV_W):
            xk_t.append(singles.tile([128, CH, w], f16, name=f"xk{i}"))
            xv_t.append(singles.tile([128, CH, w], f16, name=f"xv{i}"))
        w_sb = singles.tile([128, 3, CH, E], f16)
        b_sb = singles.tile([E, 3], f32)

        def kv_dram(which, i):
            s, m = (x_k_s, x_k_m) if which == "k" else (x_v_s, x_v_m)
            if KV_W[i] == 256:
                return s[NARROW.index(i)]
            return m[WIDE.index(i)]

        # ---- DMA issue order == arrival order == consumption order ----
        # Issued from the gpsimd SWDGE queue: its instruction stream loads
        # first (~3us), so bytes start flowing ~4us before the Sync engine
        # could even issue.
        def issue_kv(i):
            nc.gpsimd.dma_start(out=xk_t[i], in_=kv_dram("k", i))
            nc.gpsimd.dma_start(out=xv_t[i], in_=kv_dram("v", i))

        nc.gpsimd.dma_start(out=w_sb, in_=w_all[:, :, :, :])
        nc.gpsimd.dma_start(out=b_sb, in_=b_all[:, :])
        nc.gpsimd.dma_start(out=xq_t[0], in_=x_q[0])
        nc.gpsimd.dma_start(out=xq_t[1], in_=x_q[1])
        for i in range(6):
            issue_kv(i)
        nc.gpsimd.dma_start(out=xq_t[2], in_=x_q[2])
        issue_kv(6)
        nc.gpsimd.dma_start(out=xq_t[3], in_=x_q[3])
        for i in range(7, 10):
            issue_kv(i)

        bqs_sb = singles.tile([E, 1], f32)
        nc.scalar.mul(bqs_sb, b_sb[:, 0:1], SCALE)  # bq / sqrt(E)

        qT = singles.tile([E, HALF], f16)       # q^T / sqrt(E)
        kT = singles.tile([E, N], f16)          # k^T
        va = singles.tile([128, KT, MA], f16)   # v natural + two ones columns
        nc.vector.memset(va[:, :, E:], 1.0)

        # preload the Exp table off the critical path (no DMA dependency)
        warm_col = singles.tile([128, 1], f32)
        nc.vector.memset(warm_col, 0.0)
        dummy = singles.tile([128, 1], f32)
        nc.scalar.activation(dummy, warm_col, EXP)
        warm_big = singles.tile([128, BLK], f16)
        nc.vector.memset(warm_big, 0.01)

        # enough buffers to keep N_DEFER deferred exp outputs alive plus the
        # normal pipeline AND slack, so the exp stream is never in lockstep
        # with the pass-2 drain schedule through the pool ring
        pT_pool = ctx.enter_context(tc.tile_pool(name="pT", bufs=N_DEFER + 8))
        ep01 = singles.tile([MA, 2, BLK], f32)
        ep23 = singles.tile([MA, 2, BLK], f32)

        # ---- projections (chunk-granular) ----
        def proj_q(pool, j):
            """q block j (512 cols) -> qT[:, 512j:512j+512], scaled."""
            ps = pool.tile([E, BLK], f32, tag="pj", name="ps")
            for c in range(CH):
                nc.tensor.matmul(ps, lhsT=w_sb[:, 0, c, :],
                                 rhs=xq_t[j][:, c, :],
                                 start=(c == 0), stop=(c == CH - 1))
            nc.vector.tensor_scalar(
                qT[:, j * BLK:(j + 1) * BLK], ps, SCALE, bqs_sb,
                mybir.AluOpType.mult, mybir.AluOpType.add)

        def proj_k(pool, i):
            """k chunk i -> kT[:, c0:c0+w], biased."""
            c0, w = KV_COL0[i], KV_W[i]
            ps = pool.tile([E, BLK], f32, tag="pj", name="ps")
            for c in range(CH):
                nc.tensor.matmul(ps[:, :w], lhsT=w_sb[:, 1, c, :],
                                 rhs=xk_t[i][:, c, :],
                                 start=(c == 0), stop=(c == CH - 1))
            # bk is dropped: it adds a per-query-row constant to the
            # scores, which cancels exactly in the host-side normalization
            # by the ones-row sum (softmax shift invariance).
            nc.vector.tensor_copy(kT[:, c0:c0 + w], ps[:, :w])

        def proj_v(pool, kt):
            """x-stationary projection of one 128-row v tile straight into
            va[:, kt] (natural layout, no PE transpose needed)."""
            col = kt * 128
            i = max(j for j in range(len(KV_W)) if KV_COL0[j] <= col)
            sub = (col - KV_COL0[i]) // 128
            ps = pool.tile([128, E], f32, tag="pj", name="psv")
            for c in range(CH):
                nc.tensor.matmul(
                    ps, lhsT=xv_t[i][:, c, sub * 128:(sub + 1) * 128],
                    rhs=w_sb[:, 2, c, :],
                    start=(c == 0), stop=(c == CH - 1))
            nc.vector.tensor_copy(va[:, kt, 0:E], ps)

        # ---- attention stream pieces ----
        def s_exp(s_pool, u, split=False):
            blk_lo = 2 * (u // 32)
            kt = u % 32
            s2 = s_pool.tile([128, 2 * BLK], f32, tag="s", name="s2")
            for i in range(2):
                nc.tensor.matmul(
                    s2[:, i * BLK:(i + 1) * BLK],
                    lhsT=kT[:, kt * 128:(kt + 1) * 128],
                    rhs=qT[:, (blk_lo + i) * BLK:(blk_lo + i + 1) * BLK],
                    start=True, stop=True, skip_group_check=True)
            pT2 = pT_pool.tile([128, 2 * BLK], f16, tag="pT")
            if split:
                # last unit: expose block hi's exp first so the tail AV /
                # copy / DMA chain starts ~0.8us earlier
                nc.scalar.activation(pT2[:, BLK:], s2[:, BLK:], EXP)
                nc.scalar.activation(pT2[:, :BLK], s2[:, :BLK], EXP)
            else:
                nc.scalar.activation(pT2, s2, EXP)
            return pT2

        def av(u, pT2, oT, first, last, rev=False):
            kt = u % 32
            for i in ((1, 0) if rev else (0, 1)):
                nc.tensor.matmul(
                    oT[i],
                    lhsT=va[:, kt, :],
                    rhs=pT2[:, i * BLK:(i + 1) * BLK],
                    start=first, stop=last, skip_group_check=True)

        def epilogue(ep_sb, pair, oT_pair):
            """Copy both oT banks of a block-pair and DMA them out in one
            issue (out[2p:2p+2])."""
            nc.vector.tensor_copy(ep_sb[:, 1, :], oT_pair[1])
            nc.vector.tensor_copy(ep_sb[:, 0, :], oT_pair[0])
            nc.sync.dma_start(out=out[:, 2 * pair:2 * pair + 2, :], in_=ep_sb)

        # ---- PSUM layout: s-ring first (banks 0-3), rest hands off ----
        s_pool = ctx.enter_context(tc.tile_pool(name="s", bufs=2,
                                                space="PSUM"))

        # ================= prologue =================
        from contextlib import ExitStack as _ES

        with _ES() as pro:
            warm_ps = pro.enter_context(
                tc.tile_pool(name="warm", bufs=1, space="PSUM"))
            pjq = pro.enter_context(
                tc.tile_pool(name="pjq", bufs=2, space="PSUM"))
            wp = warm_ps.tile([128, BLK], f32, tag="w", name="wp")
            for _ in range(WARMUP_MMS):
                nc.tensor.matmul(wp, lhsT=warm_big[:, 0:128], rhs=warm_big,
                                 start=True, stop=True, skip_group_check=True)
            proj_q(pjq, 0)
            proj_q(pjq, 1)
            proj_k(pjq, 0)

        # ======== main stream: 64 units, deferred-AV rebalance ========
        # Units 0-31: q-blocks 0,1 + all streaming projections (pass 1 is
        # PE-oversubscribed).  The AV pairs of units 22-31 are deferred and
        # drained one per even unit during 32-50, where pass 2 is ACT-bound
        # and the PE has slack.  v-projections run 2 tiles per visit so the
        # chain-entry latency is amortized.
        # Task units chosen so each projection runs at the earliest unit
        # where its chunk has landed (DMA arrival-driven), leaving units
        # 25-31 as pure attention so the exp stream runs dense there.
        k_sched = {1: 0, 2: 1, 3: 4, 4: 8, 5: 11, 6: 14, 7: 18, 8: 20, 9: 21}
        # v-projection visits: (unit, first tile, n tiles).  Tiles >= 18
        # have deferred AVs, so their projections move past the k tasks
        # into the quiet units 25-30.
        v_sched = [(0, 0, 2), (2, 2, 2), (4, 4, 2), (5, 6, 2), (7, 8, 2),
                   (9, 10, 4), (12, 14, 4), (25, 18, 4), (27, 22, 4),
                   (28, 26, 4), (30, 30, 2)]
        u_defer0 = 32 - N_DEFER          # first deferred unit (22)
        with _ES() as main_sc:
            o1 = main_sc.enter_context(tc.tile_pool(name="o1", bufs=1,
                                                    space="PSUM"))
            oT01 = [o1.tile([MA, BLK], f32, tag=f"oT{i}", name=f"oT{i}")
                    for i in range(2)]
            pT_hist = {}

            with _ES() as p1:
                pj1 = p1.enter_context(tc.tile_pool(name="pj1", bufs=2,
                                                    space="PSUM"))
                pend = {}
                for i, u in k_sched.items():
                    pend.setdefault(u, []).append(lambda i=i: proj_k(pj1, i))
                for u, kt0, n in v_sched:
                    pend.setdefault(u, []).append(
                        lambda kt0=kt0, n=n: [proj_v(pj1, kt)
                                              for kt in range(kt0, kt0 + n)])
                pend.setdefault(12, []).append(lambda: proj_q(pj1, 2))
                pend.setdefault(16, []).append(lambda: proj_q(pj1, 3))

                for u in range(32):
                    pT_hist[u] = s_exp(s_pool, u)
                    for fn in pend.pop(u, ()):
                        fn()
                    if 2 <= u and u - 2 < u_defer0:
                        av(u - 2, pT_hist.pop(u - 2), oT01,
                           first=(u - 2 == 0), last=False)

            # pj1 closed -> banks 6,7 free for o2
            o2 = main_sc.enter_context(tc.tile_pool(name="o2", bufs=1,
                                                    space="PSUM"))
            oT23 = [o2.tile([MA, BLK], f32, tag=f"oT{i + 2}",
                            name=f"oT{i + 2}") for i in range(2)]

            drain_at = {du: u_defer0 + j for j, du in enumerate(
                [32, 33] + list(range(34, 34 + 2 * (N_DEFER - 2), 2)))}
            for u in range(32, 64):
                # drain deferred pass-1 AV pairs BEFORE this unit's S pair:
                # their inputs are long ready, so the PE chews them while
                # the s-ring semaphore from the exp stream resolves
                if u in drain_at:
                    du = drain_at[u]
                    av(du, pT_hist.pop(du), oT01, first=False,
                       last=(du == 31), rev=(du == 31))
                    if du == 31:
                        epilogue(ep01, 0, oT01)
                pT_hist[u] = s_exp(s_pool, u, split=(u == 63))
                if 34 <= u:
                    av(u - 2, pT_hist.pop(u - 2), oT23,
                       first=(u - 2 == 32), last=False)
            av(62, pT_hist.pop(62), oT23, first=False, last=False)
            av(63, pT_hist.pop(63), oT23, first=False, last=True, rev=True)
            # tail: block-3 copy on DVE, block-2 copy on the now-idle scalar
            # engine, one DMA issue for both
            nc.vector.tensor_copy(ep23[:, 1, :], oT23[1])
            nc.scalar.copy(ep23[:, 0, :], oT23[0])
            nc.sync.dma_start(out=out[:, 2:4, :], in_=ep23)

    nc.finalize()
    return nc


def get_nc():
    if "nc" not in _CACHE:
        _CACHE["nc"] = _build()
    return _CACHE["nc"]


def _feat_major(x2d):
    """[seq, D] fp32 -> [128, CH, seq] fp16 (feature-major, chunked)."""
    xT = np.ascontiguousarray(x2d.T)                 # [D, seq]
    xT = xT.reshape(CH, 128, -1).transpose(1, 0, 2)  # [128, CH, seq]
    return np.ascontiguousarray(xT).astype(np.float16)


def _kv_pack(fm):
    """[128, CH, 4096] -> (narrow [n,128,CH,256], wide [m,128,CH,512])."""
    nar = np.stack([fm[:, :, KV_COL0[i]:KV_COL0[i] + 256] for i in NARROW])
    wid = np.stack([fm[:, :, KV_COL0[i]:KV_COL0[i] + 512] for i in WIDE])
    return np.ascontiguousarray(nar), np.ascontiguousarray(wid)


def make_in_maps(queries, keys, values, Wq, bq, Wk, bk, Wv, bv):
    def w_prep(w):
        w = np.asarray(w, np.float32).reshape(CH, 128, E)
        return w.transpose(1, 0, 2).astype(np.float16)  # [128, CH, E]

    w_all = np.ascontiguousarray(
        np.stack([w_prep(Wq), w_prep(Wk), w_prep(Wv)], axis=1))
    b_all = np.ascontiguousarray(
        np.stack([bq, bk, bv], axis=1).astype(np.float32))
    shared = {"w_all": w_all, "b_all": b_all}

    queries = np.asarray(queries, np.float32)
    keys = np.asarray(keys, np.float32)
    values = np.asarray(values, np.float32)
    kv_cache = {}
    in_maps = []
    for c in range(NCORES):
        b, h = divmod(c, 2)
        if b not in kv_cache:
            ks, km = _kv_pack(_feat_major(keys[b]))
            vs, vm = _kv_pack(_feat_major(values[b]))
            kv_cache[b] = (ks, km, vs, vm)
        ks, km, vs, vm = kv_cache[b]
        fq = _feat_major(queries[b, h * HALF:(h + 1) * HALF, :])
        xq = np.ascontiguousarray(
            np.stack([fq[:, :, j * BLK:(j + 1) * BLK] for j in range(4)]))
        in_maps.append({
            "x_q": xq,
            "x_k_s": ks, "x_k_m": km,
            "x_v_s": vs, "x_v_m": vm,
            **shared,
        })
    return in_maps


def run(trace=False, **inputs):
    from concourse.bass_utils import run_bass_kernel_spmd

    nc = get_nc()
    in_maps = make_in_maps(**inputs)
    res = run_bass_kernel_spmd(
        nc, in_maps, core_ids=list(range(NCORES)), trace=trace)
    bv = np.asarray(inputs["bv"], np.float32)
    full = np.empty((B, N, E), dtype=np.float32)
    for c in range(NCORES):
        b, h = divmod(c, 2)
        oT = res.results[c]["out"].transpose(1, 0, 2)   # [4, MA, BLK]
        o = oT[:, :E, :] / oT[:, E:E + 1, :]            # normalize
        o = o.transpose(0, 2, 1).reshape(HALF, E) + bv  # [2048, 64]
        full[b, h * HALF:(h + 1) * HALF, :] = o
    return full, res


def kernel(**inputs):
    full, _ = run(trace=False, **inputs)
    return full


# revision 31
# speedup vs baseline: 1.0321x; 1.0055x over previous
"""Trainium2 Bass kernel for batched scaled-dot-product attention.

Problem (all fp32):
    q = queries @ Wq + bq          [B=4, N=4096, E=64]   (D_MODEL=768)
    k = keys    @ Wk + bk
    v = values  @ Wv + bv
    out = softmax(q k^T / sqrt(E)) @ v                    [B, N, 64]

Sharding: 8 cores, data-parallel over batch x query-half.  Core c handles
batch b=c//2, query rows [h*2048, (h+1)*2048) with h=c%2; it loads the full
keys/values for its batch (softmax needs every key).

Numerics (rel err ~7.5e-4):
  * fp16 everywhere on chip; inputs staged feature-major [128, CH, seq];
    q pre-scaled by 1/sqrt(E); v projected x-stationary straight into
    natural layout va [128, 32, 66] with two ones columns so row sums fall
    out of the AV matmul; outputs leave unnormalized, host divides by the
    ones-row sum, transposes and adds bv (exact: softmax weights sum to
    1); the k-projection bias is dropped entirely (it adds a per-query-row
    constant to the scores, which the ones-row normalization cancels
    exactly).

Schedule (116.4us vs the 129us baseline, rebuilt from NTFF trace analysis):
  * The old kernel was input-DMA-gated: a 6-buffer staging pool meant the
    last x chunks could not even ISSUE until t=48us, and 2KB descriptors
    made each issue cost ~1-5.6us of serial Sync-engine SWDGE time.  Now
    every chunk is host-packed chunk-contiguous ([128, CH, w] per chunk,
    one ~3-6KB descriptor per partition) and all chunks are SBUF-resident
    (18MB), so all 26 dma_starts (gpsimd SWDGE queue) issue back-to-back
    from t~8us and the 16.6MB stream runs at the full ~360GB/s, done by
    ~50us.  k/v chunks interleave in consumption order (256-col edge
    chunks, 512 in the middle); q blocks 2,3 land mid-pass.
  * 12 full-contraction warmup matmuls draw real power so the HAM clock
    gate lifts at ~12.5us (vs ~21us), putting the whole prologue
    projection chain at 2.4GHz.
  * One fused 64-unit stream (unit u: pass p=u//32, k-tile kt=u%32):
    S^T [128,1024] via 2 matmuls -> one wide exact exp on the scalar
    engine (the structural bottleneck: 64 x ~1.1us) -> 2 AV matmuls
    issued 2 units behind.  The PSUM s-ring (2x2 banks) is shared across
    both passes so the scalar engine never drains at the pass boundary;
    oT pools for pass 1/2 hand off banks mid-stream (2+2, with the 2-bank
    projection pool closed just before).
  * Pass 1 is PE-oversubscribed (all projections live there), pass 2 is
    ACT-bound with ~200ns/unit of PE slack: the last N_DEFER AV pairs of
    pass 1 are deferred (their exp outputs kept alive in SBUF) and
    drained into pass 2, front-loaded into the units that have no own-AV
    yet and issued before each unit's S pair.  Projection tasks are placed
    at the earliest unit where their chunk has landed, leaving units
    25-31 pure attention so the exp stream runs dense into the boundary.
  * Tail: the last unit's exp is split in halves (block 3 first) and the
    two epilogue copies run on DVE and the then-idle scalar engine in
    parallel, into one [MA, 2, BLK] tile -> single DMA issue per pair.
"""

import numpy as np

B, N, D, E = 4, 4096, 768, 64
NCORES = 8
HALF = N // 2          # query rows per core
CH = D // 128          # 6 feature chunks of the contraction dim
KT = N // 128          # 32 key tiles
BLK = 512              # query block (one PSUM bank of fp32)
SCALE = 1.0 / 8.0      # 1/sqrt(E)
MA = E + 2             # va stationary width (v + two ones columns)
WARMUP_MMS = 12        # K=128 N=512 dummies: real load, ramps the PE clock
                       # to 2.4GHz before the first projections
N_DEFER = 14           # pass-1 AV pairs deferred into pass-2's PE slack

# k/v chunk widths (cols of the 4096 seq): fine at the edges (fast start,
# fine-grained tail deadlines), coarse in the middle.
KV_W = [256, 256, 512, 512, 512, 512, 512, 512, 256, 256]
KV_COL0 = [0]
for _w in KV_W:
    KV_COL0.append(KV_COL0[-1] + _w)
KV_COL0 = KV_COL0[:-1]
NARROW = [i for i, w in enumerate(KV_W) if w == 256]   # -> x_*_s rows
WIDE = [i for i, w in enumerate(KV_W) if w == 512]     # -> x_*_m rows

_CACHE = {}


def _build():
    from contextlib import ExitStack

    import concourse.mybir as mybir
    import concourse.tile as tile
    from concourse import bacc

    f32 = mybir.dt.float32
    f16 = mybir.dt.float16
    EXP = mybir.ActivationFunctionType.Exp

    nc = bacc.Bacc(trn_type="TRN2")
    x_q = nc.dram_tensor("x_q", [4, 128, CH, BLK], f16, kind="ExternalInput")
    x_k_s = nc.dram_tensor("x_k_s", [len(NARROW), 128, CH, 256], f16,
                           kind="ExternalInput")
    x_k_m = nc.dram_tensor("x_k_m", [len(WIDE), 128, CH, 512], f16,
                           kind="ExternalInput")
    x_v_s = nc.dram_tensor("x_v_s", [len(NARROW), 128, CH, 256], f16,
                           kind="ExternalInput")
    x_v_m = nc.dram_tensor("x_v_m", [len(WIDE), 128, CH, 512], f16,
                           kind="ExternalInput")
    w_all = nc.dram_tensor("w_all", [128, 3, CH, E], f16, kind="ExternalInput")
    b_all = nc.dram_tensor("b_all", [E, 3], f32, kind="ExternalInput")

    out = nc.dram_tensor("out", [MA, 4, BLK], f32, kind="ExternalOutput")

    with tile.TileContext(nc) as tc, ExitStack() as ctx:
        singles = ctx.enter_context(tc.tile_pool(name="singles", bufs=1))

        # ---- resident input chunk tiles ----
        xq_t = [singles.tile([128, CH, BLK], f16, name=f"xq{j}")
                for j in range(4)]
        xk_t, xv_t = [], []
        for i, w in enumerate(KV_W):
            xk_t.append(singles.tile([128, CH, w], f16, name=f"xk{i}"))
            xv_t.append(singles.tile([128, CH, w], f16, name=f"xv{i}"))
        w_sb = singles.tile([128, 3, CH, E], f16)
        b_sb = singles.tile([E, 3], f32)

        def kv_dram(which, i):
            s, m = (x_k_s, x_k_m) if which == "k" else (x_v_s, x_v_m)
            if KV_W[i] == 256:
                return s[NARROW.index(i)]
            return m[WIDE.index(i)]

        # ---- DMA issue order == arrival order == consumption order ----
        # Issued from the gpsimd SWDGE queue: its instruction stream loads
        # first (~3us), so bytes start flowing ~4us before the Sync engine
        # could even issue.
        def issue_kv(i):
            nc.gpsimd.dma_start(out=xk_t[i], in_=kv_dram("k", i))
            nc.gpsimd.dma_start(out=xv_t[i], in_=kv_dram("v", i))

        nc.gpsimd.dma_start(out=w_sb, in_=w_all[:, :, :, :])
        nc.gpsimd.dma_start(out=b_sb, in_=b_all[:, :])
        nc.gpsimd.dma_start(out=xq_t[0], in_=x_q[0])
        nc.gpsimd.dma_start(out=xq_t[1], in_=x_q[1])
        for i in range(6):
            issue_kv(i)
        nc.gpsimd.dma_start(out=xq_t[2], in_=x_q[2])
        issue_kv(6)
        nc.gpsimd.dma_start(out=xq_t[3], in_=x_q[3])
        for i in range(7, 10):
            issue_kv(i)

        bqs_sb = singles.tile([E, 1], f32)
        nc.scalar.mul(bqs_sb, b_sb[:, 0:1], SCALE)  # bq / sqrt(E)

        qT = singles.tile([E, HALF], f16)       # q^T / sqrt(E)
        kT = singles.tile([E, N], f16)          # k^T
        va = singles.tile([128, KT, MA], f16)   # v natural + two ones columns
        nc.vector.memset(va[:, :, E:], 1.0)

        # preload the Exp table off the critical path (no DMA dependency)
        warm_col = singles.tile([128, 1], f32)
        nc.vector.memset(warm_col, 0.0)
        dummy = singles.tile([128, 1], f32)
        nc.scalar.activation(dummy, warm_col, EXP)
        warm_big = singles.tile([128, BLK], f16)
        nc.vector.memset(warm_big, 0.01)

        # enough buffers to keep N_DEFER deferred exp outputs alive plus the
        # normal pipeline AND slack, so the exp stream is never in lockstep
        # with the pass-2 drain schedule through the pool ring
        pT_pool = ctx.enter_context(tc.tile_pool(name="pT", bufs=N_DEFER + 8))
        ep01 = singles.tile([MA, 2, BLK], f32)
        ep23 = singles.tile([MA, 2, BLK], f32)

        # ---- projections (chunk-granular) ----
        def proj_q(pool, j):
            """q block j (512 cols) -> qT[:, 512j:512j+512], scaled."""
            ps = pool.tile([E, BLK], f32, tag="pj", name="ps")
            for c in range(CH):
                nc.tensor.matmul(ps, lhsT=w_sb[:, 0, c, :],
                                 rhs=xq_t[j][:, c, :],
                                 start=(c == 0), stop=(c == CH - 1))
            nc.vector.tensor_scalar(
                qT[:, j * BLK:(j + 1) * BLK], ps, SCALE, bqs_sb,
                mybir.AluOpType.mult, mybir.AluOpType.add)

        def proj_k(pool, i):
            """k chunk i -> kT[:, c0:c0+w], biased."""
            c0, w = KV_COL0[i], KV_W[i]
            ps = pool.tile([E, BLK], f32, tag="pj", name="ps")
            for c in range(CH):
                nc.tensor.matmul(ps[:, :w], lhsT=w_sb[:, 1, c, :],
                                 rhs=xk_t[i][:, c, :],
                                 start=(c == 0), stop=(c == CH - 1))
            # bk is dropped: it adds a per-query-row constant to the
            # scores, which cancels exactly in the host-side normalization
            # by the ones-row sum (softmax shift invariance).
            nc.vector.tensor_copy(kT[:, c0:c0 + w], ps[:, :w])

        def proj_v(pool, kt):
            """x-stationary projection of one 128-row v tile straight into
            va[:, kt] (natural layout, no PE transpose needed)."""
            col = kt * 128
            i = max(j for j in range(len(KV_W)) if KV_COL0[j] <= col)
            sub = (col - KV_COL0[i]) // 128
            ps = pool.tile([128, E], f32, tag="pj", name="psv")
            for c in range(CH):
                nc.tensor.matmul(
                    ps, lhsT=xv_t[i][:, c, sub * 128:(sub + 1) * 128],
                    rhs=w_sb[:, 2, c, :],
                    start=(c == 0), stop=(c == CH - 1))
            nc.vector.tensor_copy(va[:, kt, 0:E], ps)

        # ---- attention stream pieces ----
        def s_exp(s_pool, u, split=False):
            blk_lo = 2 * (u // 32)
            kt = u % 32
            s2 = s_pool.tile([128, 2 * BLK], f32, tag="s", name="s2")
            for i in range(2):
                nc.tensor.matmul(
                    s2[:, i * BLK:(i + 1) * BLK],
                    lhsT=kT[:, kt * 128:(kt + 1) * 128],
                    rhs=qT[:, (blk_lo + i) * BLK:(blk_lo + i + 1) * BLK],
                    start=True, stop=True, skip_group_check=True)
            pT2 = pT_pool.tile([128, 2 * BLK], f16, tag="pT")
            if split:
                # last unit: expose block hi's exp first so the tail AV /
                # copy / DMA chain starts ~0.8us earlier
                nc.scalar.activation(pT2[:, BLK:], s2[:, BLK:], EXP)
                nc.scalar.activation(pT2[:, :BLK], s2[:, :BLK], EXP)
            else:
                nc.scalar.activation(pT2, s2, EXP)
            return pT2

        def av(u, pT2, oT, first, last, rev=False):
            kt = u % 32
            for i in ((1, 0) if rev else (0, 1)):
                nc.tensor.matmul(
                    oT[i],
                    lhsT=va[:, kt, :],
                    rhs=pT2[:, i * BLK:(i + 1) * BLK],
                    start=first, stop=last, skip_group_check=True)

        def epilogue(ep_sb, pair, oT_pair):
            """Copy both oT banks of a block-pair and DMA them out in one
            issue (out[2p:2p+2])."""
            nc.vector.tensor_copy(ep_sb[:, 1, :], oT_pair[1])
            nc.vector.tensor_copy(ep_sb[:, 0, :], oT_pair[0])
            nc.sync.dma_start(out=out[:, 2 * pair:2 * pair + 2, :], in_=ep_sb)

        # ---- PSUM layout: s-ring first (banks 0-3), rest hands off ----
        s_pool = ctx.enter_context(tc.tile_pool(name="s", bufs=2,
                                                space="PSUM"))

        # ================= prologue =================
        from contextlib import ExitStack as _ES

        with _ES() as pro:
            warm_ps = pro.enter_context(
                tc.tile_pool(name="warm", bufs=1, space="PSUM"))
            pjq = pro.enter_context(
                tc.tile_pool(name="pjq", bufs=2, space="PSUM"))
            wp = warm_ps.tile([128, BLK], f32, tag="w", name="wp")
            for _ in range(WARMUP_MMS):
                nc.tensor.matmul(wp, lhsT=warm_big[:, 0:128], rhs=warm_big,
                                 start=True, stop=True, skip_group_check=True)
            proj_q(pjq, 0)
            proj_q(pjq, 1)
            proj_k(pjq, 0)

        # ======== main stream: 64 units, deferred-AV rebalance ========
        # Units 0-31: q-blocks 0,1 + all streaming projections (pass 1 is
        # PE-oversubscribed).  The AV pairs of units 22-31 are deferred and
        # drained one per even unit during 32-50, where pass 2 is ACT-bound
        # and the PE has slack.  v-projections run 2 tiles per visit so the
        # chain-entry latency is amortized.
        # Task units chosen so each projection runs at the earliest unit
        # where its chunk has landed (DMA arrival-driven), leaving units
        # 25-31 as pure attention so the exp stream runs dense there.
        k_sched = {1: 0, 2: 1, 3: 4, 4: 8, 5: 11, 6: 14, 7: 18, 8: 20, 9: 21}
        # v-projection visits: (unit, first tile, n tiles).  Tiles >= 18
        # have deferred AVs, so their projections move past the k tasks
        # into the quiet units 25-30.
        v_sched = [(0, 0, 2), (2, 2, 2), (4, 4, 2), (5, 6, 2), (7, 8, 2),
                   (9, 10, 4), (12, 14, 4), (25, 18, 4), (27, 22, 4),
                   (28, 26, 4), (30, 30, 2)]
        u_defer0 = 32 - N_DEFER          # first deferred unit (22)
        with _ES() as main_sc:
            o1 = main_sc.enter_context(tc.tile_pool(name="o1", bufs=1,
                                                    space="PSUM"))
            oT01 = [o1.tile([MA, BLK], f32, tag=f"oT{i}", name=f"oT{i}")
                    for i in range(2)]
            pT_hist = {}

            with _ES() as p1:
                pj1 = p1.enter_context(tc.tile_pool(name="pj1", bufs=2,
                                                    space="PSUM"))
                pend = {}
                for i, u in k_sched.items():
                    pend.setdefault(u, []).append(lambda i=i: proj_k(pj1, i))
                for u, kt0, n in v_sched:
                    pend.setdefault(u, []).append(
                        lambda kt0=kt0, n=n: [proj_v(pj1, kt)
                                              for kt in range(kt0, kt0 + n)])
                pend.setdefault(12, []).append(lambda: proj_q(pj1, 2))
                pend.setdefault(16, []).append(lambda: proj_q(pj1, 3))

                for u in range(32):
                    pT_hist[u] = s_exp(s_pool, u)
                    for fn in pend.pop(u, ()):
                        fn()
                    if 2 <= u and u - 2 < u_defer0:
                        av(u - 2, pT_hist.pop(u - 2), oT01,
                           first=(u - 2 == 0), last=False)

            # pj1 closed -> banks 6,7 free for o2
            o2 = main_sc.enter_context(tc.tile_pool(name="o2", bufs=1,
                                                    space="PSUM"))
            oT23 = [o2.tile([MA, BLK], f32, tag=f"oT{i + 2}",
                            name=f"oT{i + 2}") for i in range(2)]

            drain_at = {du: u_defer0 + j for j, du in enumerate(
                [32, 33] + list(range(34, 34 + 2 * (N_DEFER - 2), 2)))}
            for u in range(32, 64):
                # drain deferred pass-1 AV pairs BEFORE this unit's S pair:
                # their inputs are long ready, so the PE chews them while
                # the s-ring semaphore from the exp stream resolves
                if u in drain_at:
                    du = drain_at[u]
                    av(du, pT_hist.pop(du), oT01, first=False,
                       last=(du == 31), rev=(du == 31))
                    if du == 31:
                        epilogue(ep01, 0, oT01)
                pT_hist[u] = s_exp(s_pool, u, split=(u == 63))
                if 34 <= u:
                    av(u - 2, pT_hist.pop(u - 2), oT23,
                       first=(u - 2 == 32), last=False)
            av(62, pT_hist.pop(62), oT23, first=False, last=False)
            av(63, pT_hist.pop(63), oT23, first=False, last=True, rev=True)
            # tail: block-3 copy on DVE, block-2 copy on the now-idle scalar
            # engine, one DMA issue for both
            nc.vector.tensor_copy(ep23[:, 1, :], oT23[1])
            nc.scalar.copy(ep23[:, 0, :], oT23[0])
            nc.sync.dma_start(out=out[:, 2:4, :], in_=ep23)

    nc.finalize()
    return nc


def get_nc():
    if "nc" not in _CACHE:
        _CACHE["nc"] = _build()
    return _CACHE["nc"]


def _feat_major(x2d):
    """[seq, D] fp32 -> [128, CH, seq] fp16 (feature-major, chunked)."""
    xT = np.ascontiguousarray(x2d.T)                 # [D, seq]
    xT = xT.reshape(CH, 128, -1).transpose(1, 0, 2)  # [128, CH, seq]
    return np.ascontiguousarray(xT).astype(np.float16)


def _kv_pack(fm):
    """[128, CH, 4096] -> (narrow [n,128,CH,256], wide [m,128,CH,512])."""
    nar = np.stack([fm[:, :, KV_COL0[i]:KV_COL0[i] + 256] for i in NARROW])
    wid = np.stack([fm[:, :, KV_COL0[i]:KV_COL0[i] + 512] for i in WIDE])
    return np.ascontiguousarray(nar), np.ascontiguousarray(wid)


def make_in_maps(queries, keys, values, Wq, bq, Wk, bk, Wv, bv):
    def w_prep(w):
        w = np.asarray(w, np.float32).reshape(CH, 128, E)
        return w.transpose(1, 0, 2).astype(np.float16)  # [128, CH, E]

    w_all = np.ascontiguousarray(
        np.stack([w_prep(Wq), w_prep(Wk), w_prep(Wv)], axis=1))
    b_all = np.ascontiguousarray(
        np.stack([bq, bk, bv], axis=1).astype(np.float32))
    shared = {"w_all": w_all, "b_all": b_all}

    queries = np.asarray(queries, np.float32)
    keys = np.asarray(keys, np.float32)
    values = np.asarray(values, np.float32)
    kv_cache = {}
    in_maps = []
    for c in range(NCORES):
        b, h = divmod(c, 2)
        if b not in kv_cache:
            ks, km = _kv_pack(_feat_major(keys[b]))
            vs, vm = _kv_pack(_feat_major(values[b]))
            kv_cache[b] = (ks, km, vs, vm)
        ks, km, vs, vm = kv_cache[b]
        fq = _feat_major(queries[b, h * HALF:(h + 1) * HALF, :])
        xq = np.ascontiguousarray(
            np.stack([fq[:, :, j * BLK:(j + 1) * BLK] for j in range(4)]))
        in_maps.append({
            "x_q": xq,
            "x_k_s": ks, "x_k_m": km,
            "x_v_s": vs, "x_v_m": vm,
            **shared,
        })
    return in_maps


def run(trace=False, **inputs):
    from concourse.bass_utils import run_bass_kernel_spmd

    nc = get_nc()
    in_maps = make_in_maps(**inputs)
    res = run_bass_kernel_spmd(
        nc, in_maps, core_ids=list(range(NCORES)), trace=trace)
    bv = np.asarray(inputs["bv"], np.float32)
    full = np.empty((B, N, E), dtype=np.float32)
    for c in range(NCORES):
        b, h = divmod(c, 2)
        oT = res.results[c]["out"].transpose(1, 0, 2)   # [4, MA, BLK]
        o = oT[:, :E, :] / oT[:, E:E + 1, :]            # normalize
        o = o.transpose(0, 2, 1).reshape(HALF, E) + bv  # [2048, 64]
        full[b, h * HALF:(h + 1) * HALF, :] = o
    return full, res


def kernel(**inputs):
    full, _ = run(trace=False, **inputs)
    return full
